# revision 1
# baseline (speedup 1.0000x reference)
"""Trainium2 Bass kernel for nn_DetectionHead (NMS detection head), v2.

Computes, for x[8, 2048, 2048] f32:
    xp  = relu(x - eps)
    xm  = 3x3 hole-excluded neighborhood max of xp (zero padding)
    out = xp * (x > xm)

Sharding: batch (8 images) across the 8 NeuronCores, data parallel.  The
host pads each image with a 1-pixel zero border ([2050, 2050]) and also
ships an fp16 copy of the padded image.

Strategy (baseline all-f32 was ~292us):
 - The 3x3 pool runs in fp16 on the DVE: 2-byte packed operands engage the
   DVE 2x datapath mode (fp16 max is exact - it only selects inputs), so the
   4 pool layers cost half their f32 time.
 - The center comparison keeps full f32 precision: e = (m16 - eps) < x_f32
   (mixed-dtype STT).  Only the neighborhood max is fp16-rounded; measured
   rel-err vs the f32 reference on the real input is 1.41e-2 (mask flips at
   near-ties), within the 2e-2 gate, and bit-identical to an offline numpy
   simulation of this exact pipeline.
 - The f32->fp16 center cast and the value path relu(x-eps) run on the
   otherwise-idle Activation engine.
 - Loads: f32 center rows once (16.8MB) + the 2 fp16 halo rows per
   partition straight from the host fp16 image (4.2MB) - vs 25.2MB for
   f32-with-halo.  Stores are fp16 (8.4MB), host upcasts.
 - xp and e live in one 64B-aligned-pitch tile so the final mult gets the
   fast DVE path.

Per-core pipeline, full-width row bands (band t = image rows [512t, 512t+512),
partition p covers padded rows 512t+4p .. 512t+4p+5):

    SP   x16 rows {0,5}  <- fp16 halo rows from host image      (DMA)
    SP   xtc [P,4,2050]  <- f32 center rows                     (DMA)
    ACT  x16 rows 1..4   <- fp16(xtc)
    ACT  xpe rows 0..3   <- relu(xtc.center - eps)  (fp16 out)
    DVE  v   = max(x16[0:4], x16[2:6])      vertical hole pair
    DVE  c   = max(v, x16[1:5])             -> in-place onto x16[1:5]
    DVE  t   = max(c@col-1, c@col+1)        -> x16[0:4]  (trailing write)
    DVE  m   = max(t, v@center)             in-place
    DVE  e   = (m - eps) < xtc              -> xpe rows 4..7
    DVE  out = xpe[0:4] * xpe[4:8]          -> o
    SP   store o -> dense [H, W] fp16 DRAM (contiguous 16KB/partition)

Band 0 is column-split so the DVE starts after ~half the first load+cast;
the last band's e/out/store are column-split so the store overlaps compute.
"""

import numpy as np

import concourse.bacc as bacc
import concourse.mybir as mybir
import concourse.tile as tile
from concourse import bass_utils
from concourse.ap import AP

EPS = 0.01
B, H, W = 8, 2048, 2048
HP2, WP2 = H + 2, W + 2
P = 128
RB = 4
BAND_H = RB * P           # 512
NBAND = H // BAND_H       # 4
SB = RB + 2
HL = W // 2               # 1024
F32 = mybir.dt.float32
F16 = mybir.dt.float16
MX = mybir.AluOpType.max
SUB = mybir.AluOpType.subtract
LT = mybir.AluOpType.is_lt


def _rowsel(tile_, rows, width=WP2):
    """AP over `tile_` selecting a strided row subset per partition."""
    ap = tile_[:]
    part = list(ap.ap[0])
    r0 = rows[0]
    stride = (rows[1] - rows[0]) if len(rows) > 1 else 1
    return AP(ap.tensor, ap.offset + r0 * WP2, [part, [stride * WP2, len(rows)], [1, width]])


def _emit_pipeline(nc, tc, x_d, xh_d, o_d, out_row_stride, out_offset0, mode="full"):
    do_load = mode in ("full", "dmaonly", "loadonly")
    do_store = mode in ("full", "dmaonly", "storeonly")
    do_compute = mode in ("full", "nodma")
    with (
        tc.tile_pool(name="ioc", bufs=2) as ioc,
        tc.tile_pool(name="p16", bufs=2) as p16,
        tc.tile_pool(name="wv", bufs=1) as wv,
        tc.tile_pool(name="wpe", bufs=1) as wpe,
        tc.tile_pool(name="ioo", bufs=1) as ioo,
        tc.tile_pool(name="cst", bufs=1) as cst,
    ):
        negeps = cst.tile([P, 1], F32, tag="negeps", name="negeps")
        nc.vector.memset(negeps[:], -EPS)
        for t in range(NBAND):
            first, last = (t == 0), (t == NBAND - 1)
            xtc = ioc.tile([P, RB, WP2], F32, tag="xtc", name="xtc")
            x16 = p16.tile([P, SB, WP2], F16, tag="x16", name="x16")
            v = wv.tile([P, RB, WP2], F16, tag="v", name="v")
            xpe = wpe.tile([P, 2 * RB, W], F16, tag="xpe", name="xpe")
            o = ioo.tile([P, RB, W], F16, tag="o", name="o")

            if do_load:
                # fp16 halo rows (padded rows 512t+4p and 512t+4p+5) into
                # x16 rows 0 and 5
                nc.sync.dma_start(
                    out=_rowsel(x16, (0, 5)),
                    in_=AP(
                        xh_d.tensor,
                        t * BAND_H * WP2,
                        [[RB * WP2, P], [5 * WP2, 2], [1, WP2]],
                    ),
                )
                # f32 center rows 512t+4p+1 .. +4
                if first and do_compute:
                    for c0, cw in ((0, HL + 2), (HL + 2, WP2 - HL - 2)):
                        nc.sync.dma_start(
                            out=xtc[:, :, c0 : c0 + cw],
                            in_=AP(
                                x_d.tensor,
                                (t * BAND_H + 1) * WP2 + c0,
                                [[RB * WP2, P], [WP2, RB], [1, cw]],
                            ),
                        )
                else:
                    nc.sync.dma_start(
                        out=xtc[:],
                        in_=AP(
                            x_d.tensor,
                            (t * BAND_H + 1) * WP2,
                            [[RB * WP2, P], [WP2, RB], [1, WP2]],
                        ),
                    )
            else:
                nc.gpsimd.memset(xtc[:], 0.25)
                nc.vector.memset(_rowsel(x16, (0, 5)), 0.25)

            if do_compute:
                if first:
                    cast_cols = [(0, HL + 2), (HL + 2, WP2 - HL - 2)]
                    chain_cols = [(0, HL + 2), (HL, WP2 - HL)]
                else:
                    cast_cols = [(0, WP2)]
                    chain_cols = [(0, WP2)]

                for c0, cw in cast_cols:
                    nc.scalar.activation(
                        out=x16[:, 1 : RB + 1, c0 : c0 + cw],
                        in_=xtc[:, :, c0 : c0 + cw],
                        func=mybir.ActivationFunctionType.Copy,
                    )
                xp_cols = [(0, HL), (HL, W - HL)] if first else [(0, W)]
                for c0, cw in xp_cols:
                    nc.scalar.activation(
                        out=xpe[:, 0:RB, c0 : c0 + cw],
                        in_=xtc[:, :, 1 + c0 : 1 + c0 + cw],
                        func=mybir.ActivationFunctionType.Relu, bias=negeps[:],
                    )
                # DVE fp16 pool; c onto x16[1:5] (idempotent at band-0 seam),
                # t/m onto x16[0:4] (write row trails read rows)
                for c0, cw in chain_cols:
                    nc.vector.tensor_tensor(
                        out=v[:, :, c0 : c0 + cw],
                        in0=x16[:, 0:RB, c0 : c0 + cw],
                        in1=x16[:, 2:SB, c0 : c0 + cw], op=MX
                    )
                for c0, cw in chain_cols:
                    nc.vector.tensor_tensor(
                        out=x16[:, 1 : RB + 1, c0 : c0 + cw],
                        in0=v[:, :, c0 : c0 + cw],
                        in1=x16[:, 1 : RB + 1, c0 : c0 + cw], op=MX
                    )
                for c0, cw in chain_cols:
                    ow = cw - 2
                    nc.vector.tensor_tensor(
                        out=x16[:, 0:RB, c0 : c0 + ow],
                        in0=x16[:, 1 : RB + 1, c0 : c0 + ow],
                        in1=x16[:, 1 : RB + 1, c0 + 2 : c0 + cw], op=MX
                    )
                    nc.vector.tensor_tensor(
                        out=x16[:, 0:RB, c0 : c0 + ow],
                        in0=x16[:, 0:RB, c0 : c0 + ow],
                        in1=v[:, :, c0 + 1 : c0 + ow + 1], op=MX
                    )
                ecols = [(0, HL), (HL, HL)] if (first or last) else [(0, W)]
                for c0, ow in ecols:
                    nc.vector.scalar_tensor_tensor(
                        out=xpe[:, RB : 2 * RB, c0 : c0 + ow],
                        in0=x16[:, 0:RB, c0 : c0 + ow], scalar=EPS,
                        in1=xtc[:, :, 1 + c0 : 1 + c0 + ow],
                        op0=SUB, op1=LT
                    )
                    nc.vector.tensor_tensor(
                        out=o[:, :, c0 : c0 + ow],
                        in0=xpe[:, 0:RB, c0 : c0 + ow],
                        in1=xpe[:, RB : 2 * RB, c0 : c0 + ow],
                        op=mybir.AluOpType.mult
                    )
                    if do_store and last:
                        nc.sync.dma_start(
                            out=AP(
                                o_d.tensor,
                                out_offset0 + t * BAND_H * out_row_stride + c0,
                                [[RB * out_row_stride, P],
                                 [out_row_stride, RB], [1, ow]],
                            ),
                            in_=o[:, :, c0 : c0 + ow],
                        )
            elif do_store:
                nc.gpsimd.memset(o[:], 0.25)

            if do_store and not (do_compute and last):
                nc.sync.dma_start(
                    out=AP(
                        o_d.tensor,
                        out_offset0 + t * BAND_H * out_row_stride,
                        [[RB * out_row_stride, P], [out_row_stride, RB], [1, W]],
                    ),
                    in_=o[:],
                )


def _build_program():
    nc = bacc.Bacc(
        "TRN2",
        target_bir_lowering=False,
        debug=False,
        enable_asserts=False,
        num_devices=B,
    )
    x_d = nc.dram_tensor("x", [HP2, WP2], F32, kind="ExternalInput").ap()
    xh_d = nc.dram_tensor("xh", [HP2, WP2], F16, kind="ExternalInput").ap()
    o_d = nc.dram_tensor("out", [H, W], F16, kind="ExternalOutput").ap()
    with tile.TileContext(nc) as tc:
        _emit_pipeline(nc, tc, x_d, xh_d, o_d, W, 0)
    nc.compile()
    return nc


def _build_timing_program(niter=1, mode="full"):
    """Same pipeline repeated `niter` times by a device-side loop against
    Internal DRAM scratch, with tiny external I/O so transfers are ~free.
    (wall(n2) - wall(n1)) / (n2 - n1) isolates per-pass device time."""
    nc = bacc.Bacc(
        "TRN2",
        target_bir_lowering=False,
        debug=False,
        enable_asserts=False,
        num_devices=B,
    )
    di = nc.dram_tensor("x", [1, 8], F32, kind="ExternalInput").ap()
    do = nc.dram_tensor("out", [1, 8], F32, kind="ExternalOutput").ap()
    x_d = nc.dram_tensor("xi", [HP2, WP2], F32, kind="Internal").ap()
    xh_d = nc.dram_tensor("xhi", [HP2, WP2], F16, kind="Internal").ap()
    o_d = nc.dram_tensor("oi", [H, W], F16, kind="Internal").ap()
    with tile.TileContext(nc) as tc:
        with tc.tile_pool(name="dummy", bufs=1) as dp:
            dt = dp.tile([1, 8], F32, tag="dummy")
            nc.sync.dma_start(out=dt[:], in_=di[:])
            nc.sync.dma_start(out=do[:], in_=dt[:])
        if niter == 1:
            _emit_pipeline(nc, tc, x_d, xh_d, o_d, W, 0, mode)
        else:
            with tc.For_i(0, niter, 1):
                _emit_pipeline(nc, tc, x_d, xh_d, o_d, W, 0, mode)
    nc.compile()
    return nc


_NC = None


def _get_program():
    global _NC
    if _NC is None:
        _NC = _build_program()
    return _NC


def kernel(x: np.ndarray) -> np.ndarray:
    x = np.asarray(x, dtype=np.float32)
    assert x.shape == (B, H, W), x.shape
    xpad = np.zeros((B, HP2, WP2), dtype=np.float32)
    xpad[:, 1 : H + 1, 1 : W + 1] = x
    xh = xpad.astype(np.float16)
    nc = _get_program()
    in_maps = [{"x": xpad[i], "xh": xh[i]} for i in range(B)]
    res = bass_utils.run_bass_kernel_spmd(nc, in_maps, core_ids=list(range(B)))
    return np.stack(
        [np.asarray(r["out"], dtype=np.float32) for r in res.results], axis=0
    )



# revision 5
# speedup vs baseline: 1.0324x; 1.0324x over previous
"""Trainium2 Bass kernel for nn_DetectionHead (NMS detection head), v3.

Computes, for x[8, 2048, 2048] f32:
    xp  = relu(x - eps)
    xm  = 3x3 hole-excluded neighborhood max of xp (zero padding)
    out = xp * (x > xm)

Sharding: batch (8 images) across the 8 NeuronCores, data parallel.  The
host pads each image with a 1-pixel zero border and converts to fp16
([2050, 2050]); only the fp16 image is shipped (8.4MB loads vs 21MB for
the old f32-center scheme).

Math restructure (exactly equivalent to the f32-compare variant on this
input; rel err 1.603e-2 vs the f32 reference, gate 2e-2):
 - Inclusive 3x3 max m9 replaces the hole-excluded max m8: since eps>0,
   x > m8 - eps  <=>  x > m9 - eps  (m9 = max(m8, x)).  The inclusive max
   is separable: v2=max(up,down); v3=max(v2,x); t=max(v3_L,v3_R);
   m9=max(t,v3_C) - and v3 overwrites v2 in place, x16 stays intact.
 - The compare+mask-multiply tail is  out = min(xp, relu(BIG*d + BIG*eps))
   with d = fp16(x - m9) <= 0 (m9 >= x always).  d is Sterbenz-exact near
   the threshold, so the fp16 rounding of d adds no flips (verified 0
   flips vs the f32 compare).  BIG=2^22 makes the relu arm saturate to
   >=215 whenever the mask is true (max xp is ~5.2), so min() selects xp
   exactly.  This moves the mask math onto the Activation engine.

Engine budget per band (cost model, [128,4,2050] ops):
 - DVE   (2x fp16): v3, m9, d, min           = 4 x 4.33us = 17.3us
 - Pool  (gpsimd):  v2, t                    = 2 x 6.83us = 13.7us
 - ACT:             xp=relu(x-eps), h=relu() = 2 x 7.01us = 14.0us
 - DMA:   load 6 rows/part fp16 (9.5us, SP queue), store (6.3us, ACT queue)

Per-core pipeline, full-width row bands (band t = image rows [512t, 512t+512),
partition p covers padded rows 512t+4p .. 512t+4p+5).
"""

import numpy as np

import concourse.bacc as bacc
import concourse.mybir as mybir
import concourse.tile as tile
from concourse import bass_utils
from concourse.ap import AP

EPS = 0.01
B, H, W = 8, 2048, 2048
HP2, WP2 = H + 2, W + 2
P = 128
RB = 4
BAND_H = RB * P           # 512
NBAND = H // BAND_H       # 4
SB = RB + 2               # 6
F32 = mybir.dt.float32
F16 = mybir.dt.float16
MX = mybir.AluOpType.max
MN = mybir.AluOpType.min
SUB = mybir.AluOpType.subtract
RELU = mybir.ActivationFunctionType.Relu
BIG = float(2.0 ** 22)
BIGEPS = float(np.float32(0.01) * np.float32(BIG))


def _emit_pipeline(nc, tc, xh_d, o_d, out_row_stride, out_offset0, mode="full"):
    do_load = mode in ("full", "dmaonly", "loadonly")
    do_store = mode in ("full", "dmaonly", "storeonly")
    do_compute = mode in ("full", "nodma")
    with (
        tc.tile_pool(name="io16", bufs=2) as io16,
        tc.tile_pool(name="wv", bufs=2) as wv,
        tc.tile_pool(name="wt", bufs=2) as wt,
        tc.tile_pool(name="wpe", bufs=2) as wpe,
        tc.tile_pool(name="cst", bufs=1) as cst,
    ):
        negeps = cst.tile([P, 1], F32, tag="negeps", name="negeps")
        bigeps = cst.tile([P, 1], F32, tag="bigeps", name="bigeps")
        nc.vector.memset(negeps[:], -EPS)
        nc.vector.memset(bigeps[:], BIGEPS)
        for t in range(NBAND):
            x16 = io16.tile([P, SB, WP2], F16, tag="x16", name="x16")
            v = wv.tile([P, RB, WP2], F16, tag="v", name="v")
            tm = wt.tile([P, RB, W], F16, tag="tm", name="tm")
            xpe = wpe.tile([P, 2 * RB, W], F16, tag="xpe", name="xpe")

            if do_load:
                # padded rows 512t+4p .. 512t+4p+5, full 2050 width
                nc.sync.dma_start(
                    out=x16[:],
                    in_=AP(
                        xh_d.tensor,
                        t * BAND_H * WP2,
                        [[RB * WP2, P], [WP2, SB], [1, WP2]],
                    ),
                )
            elif t < 2:
                nc.vector.memset(x16[:], 0.25)

            if do_compute:
                # ACT: xp = relu(x - eps) -> xpe rows 0..RB
                nc.scalar.activation(
                    out=xpe[:, 0:RB, :],
                    in_=x16[:, 1 : RB + 1, 1 : W + 1],
                    func=RELU, bias=negeps[:],
                )
                # DVE: v2 = max(up, down)
                nc.vector.tensor_tensor(
                    out=v[:], in0=x16[:, 0:RB, :], in1=x16[:, 2:SB, :], op=MX
                )
                # DVE: v3 = max(v2, center), in place
                nc.vector.tensor_tensor(
                    out=v[:], in0=v[:], in1=x16[:, 1 : RB + 1, :], op=MX
                )
                # DVE: t = max(v3_L, v3_R)
                nc.vector.tensor_tensor(
                    out=tm[:], in0=v[:, :, 0:W], in1=v[:, :, 2:WP2], op=MX
                )
                # DVE: m9 = max(t, v3_C), in place
                nc.vector.tensor_tensor(
                    out=tm[:], in0=tm[:], in1=v[:, :, 1 : W + 1], op=MX
                )
                # DVE: d = x - m9 (<= 0), in place onto tm
                nc.vector.tensor_tensor(
                    out=tm[:], in0=x16[:, 1 : RB + 1, 1 : W + 1], in1=tm[:], op=SUB
                )
                # ACT: h = relu(BIG*d + BIG*eps) -> xpe rows RB..2RB
                nc.scalar.activation(
                    out=xpe[:, RB : 2 * RB, :], in_=tm[:],
                    func=RELU, bias=bigeps[:], scale=BIG,
                )
                # DVE: out = min(xp, h), in place onto xpe rows 0..RB
                nc.vector.tensor_tensor(
                    out=xpe[:, 0:RB, :],
                    in0=xpe[:, 0:RB, :],
                    in1=xpe[:, RB : 2 * RB, :], op=MN,
                )
            elif do_store and t < 2:
                nc.gpsimd.memset(xpe[:, 0:RB, :], 0.25)

            if do_store:
                # store on the ACT engine's DMA queue so loads/stores overlap
                nc.scalar.dma_start(
                    out=AP(
                        o_d.tensor,
                        out_offset0 + t * BAND_H * out_row_stride,
                        [[RB * out_row_stride, P], [out_row_stride, RB], [1, W]],
                    ),
                    in_=xpe[:, 0:RB, :],
                )


def _build_program():
    nc = bacc.Bacc(
        "TRN2",
        target_bir_lowering=False,
        debug=False,
        enable_asserts=False,
        num_devices=B,
    )
    xh_d = nc.dram_tensor("xh", [HP2, WP2], F16, kind="ExternalInput").ap()
    o_d = nc.dram_tensor("out", [H, W], F16, kind="ExternalOutput").ap()
    with tile.TileContext(nc) as tc:
        _emit_pipeline(nc, tc, xh_d, o_d, W, 0)
    nc.compile()
    return nc


def _build_timing_program(niter=1, mode="full"):
    """Same pipeline repeated `niter` times by a device-side loop against
    Internal DRAM scratch, with tiny external I/O so transfers are ~free.
    (wall(n2) - wall(n1)) / (n2 - n1) isolates per-pass device time."""
    nc = bacc.Bacc(
        "TRN2",
        target_bir_lowering=False,
        debug=False,
        enable_asserts=False,
        num_devices=B,
    )
    di = nc.dram_tensor("x", [1, 8], F32, kind="ExternalInput").ap()
    do = nc.dram_tensor("out", [1, 8], F32, kind="ExternalOutput").ap()
    xh_d = nc.dram_tensor("xhi", [HP2, WP2], F16, kind="Internal").ap()
    o_d = nc.dram_tensor("oi", [H, W], F16, kind="Internal").ap()
    with tile.TileContext(nc) as tc:
        with tc.tile_pool(name="dummy", bufs=1) as dp:
            dt = dp.tile([1, 8], F32, tag="dummy")
            nc.sync.dma_start(out=dt[:], in_=di[:])
            nc.sync.dma_start(out=do[:], in_=dt[:])
        if niter == 1:
            _emit_pipeline(nc, tc, xh_d, o_d, W, 0, mode)
        else:
            with tc.For_i(0, niter, 1):
                _emit_pipeline(nc, tc, xh_d, o_d, W, 0, mode)
    nc.compile()
    return nc


_NC = None


def _get_program():
    global _NC
    if _NC is None:
        _NC = _build_program()
    return _NC


def kernel(x: np.ndarray) -> np.ndarray:
    x = np.asarray(x, dtype=np.float32)
    assert x.shape == (B, H, W), x.shape
    xh = np.zeros((B, HP2, WP2), dtype=np.float16)
    xh[:, 1 : H + 1, 1 : W + 1] = x.astype(np.float16)
    nc = _get_program()
    in_maps = [{"xh": xh[i]} for i in range(B)]
    res = bass_utils.run_bass_kernel_spmd(nc, in_maps, core_ids=list(range(B)))
    return np.stack(
        [np.asarray(r["out"], dtype=np.float32) for r in res.results], axis=0
    )


# revision 6
# speedup vs baseline: 1.1498x; 1.1138x over previous
"""Trainium2 Bass kernel for nn_DetectionHead (NMS detection head), v4.

Computes, for x[8, 2048, 2048] f32:
    xp  = relu(x - eps)
    xm  = 3x3 hole-excluded neighborhood max of xp (zero padding)
    out = xp * (x > xm)

Sharding: batch (8 images) across the 8 NeuronCores, data parallel.  The
host pads each image with a 1-pixel zero border and converts to fp16
([2050, 2050]); only the fp16 image is shipped.

Math restructure (rel err 1.603e-2 vs the f32 reference, gate 2e-2):
 - Inclusive 3x3 max m9 replaces the hole-excluded max m8: since eps>0,
   x > m8 - eps  <=>  x > m9 - eps  (m9 = max(m8, x)).  Separable:
   v2=max(up,down); v3=max(v2,x) in place; t=max(v3_L,v3_R);
   m9=max(t,v3_C) in place; x16 stays intact for the compare.
 - Tail:  out = min(xp, relu(BIG*d + BIG*eps)),  d = fp16(x - m9) <= 0.
   d is Sterbenz-exact near the threshold (0 mask flips vs f32 compare);
   BIG=2^22 saturates the relu arm to >=215 when the mask is true
   (max xp ~5.2), so min() selects xp exactly.  The mask compare runs on
   the Activation engine (relu w/ scale+bias), not the DVE.

Engine budget per band ([128,4,~2050] ops, cost model):
 - DVE  (2x fp16): v2, v3, t, m9, d, min = 6 x 4.33us = 26.0us
 - ACT:            xp, h                 = 2 x 7.01us = 14.0us
 - DMA: load 6 rows/part fp16 (9.5us, SP queue), store (6.3us, Pool queue)
Band 0 is column-split so compute starts after half the load; the last
band's tail (d/h/min/store) is column-split so the store overlaps compute.

Per-core pipeline, full-width row bands (band t = image rows [512t, 512t+512),
partition p covers padded rows 512t+4p .. 512t+4p+5).
"""

import numpy as np

import concourse.bacc as bacc
import concourse.mybir as mybir
import concourse.tile as tile
from concourse import bass_utils
from concourse.ap import AP

EPS = 0.01
B, H, W = 8, 2048, 2048
HP2, WP2 = H + 2, W + 2
P = 128
RB = 4
BAND_H = RB * P           # 512
NBAND = H // BAND_H       # 4
SB = RB + 2               # 6
HL = W // 2               # 1024
F32 = mybir.dt.float32
F16 = mybir.dt.float16
MX = mybir.AluOpType.max
MN = mybir.AluOpType.min
SUB = mybir.AluOpType.subtract
RELU = mybir.ActivationFunctionType.Relu
BIG = float(2.0 ** 22)
BIGEPS = float(np.float32(0.01) * np.float32(BIG))


def _emit_pipeline(nc, tc, xh_d, o_d, out_row_stride, out_offset0, mode="full"):
    do_load = mode in ("full", "dmaonly", "loadonly")
    do_store = mode in ("full", "dmaonly", "storeonly")
    do_compute = mode in ("full", "nodma")
    with (
        tc.tile_pool(name="io16", bufs=2) as io16,
        tc.tile_pool(name="wv", bufs=2) as wv,
        tc.tile_pool(name="wt", bufs=2) as wt,
        tc.tile_pool(name="wpe", bufs=2) as wpe,
        tc.tile_pool(name="cst", bufs=1) as cst,
    ):
        negeps = cst.tile([P, 1], F32, tag="negeps", name="negeps")
        bigeps = cst.tile([P, 1], F32, tag="bigeps", name="bigeps")
        nc.vector.memset(negeps[:], -EPS)
        nc.vector.memset(bigeps[:], BIGEPS)
        for t in range(NBAND):
            first, last = (t == 0), (t == NBAND - 1)
            x16 = io16.tile([P, SB, WP2], F16, tag="x16", name="x16")
            v = wv.tile([P, RB, WP2], F16, tag="v", name="v")
            tm = wt.tile([P, RB, W], F16, tag="tm", name="tm")
            xpe = wpe.tile([P, 2 * RB, W], F16, tag="xpe", name="xpe")

            if do_load:
                # padded rows 512t+4p .. 512t+4p+5
                load_segs = [(0, HL + 2), (HL + 2, WP2)] if first else [(0, WP2)]
                for c0, c1 in load_segs:
                    nc.sync.dma_start(
                        out=x16[:, :, c0:c1],
                        in_=AP(
                            xh_d.tensor,
                            t * BAND_H * WP2 + c0,
                            [[RB * WP2, P], [WP2, SB], [1, c1 - c0]],
                        ),
                    )
            elif do_compute or t < 2:
                nc.gpsimd.memset(x16[:], 0.25)

            if do_compute:
                # segments: v2/v3 cols (padded), m9/t + xp out cols (image),
                # tail out cols (image)
                if first:
                    vsegs = [(0, HL + 2), (HL, WP2)]
                    msegs = [(0, HL), (HL, W)]
                else:
                    vsegs = [(0, WP2)]
                    msegs = [(0, W)]
                tailsegs = [(0, HL), (HL, W)] if (first or last) else [(0, W)]

                for c0, c1 in msegs:
                    # ACT: xp = relu(x - eps) -> xpe rows 0..RB
                    nc.scalar.activation(
                        out=xpe[:, 0:RB, c0:c1],
                        in_=x16[:, 1 : RB + 1, 1 + c0 : 1 + c1],
                        func=RELU, bias=negeps[:],
                    )
                for c0, c1 in vsegs:
                    # DVE: v2 = max(up, down)
                    nc.vector.tensor_tensor(
                        out=v[:, :, c0:c1],
                        in0=x16[:, 0:RB, c0:c1],
                        in1=x16[:, 2:SB, c0:c1], op=MX,
                    )
                    # DVE: v3 = max(v2, center), in place
                    nc.vector.tensor_tensor(
                        out=v[:, :, c0:c1],
                        in0=v[:, :, c0:c1],
                        in1=x16[:, 1 : RB + 1, c0:c1], op=MX,
                    )
                for c0, c1 in msegs:
                    # DVE: t = max(v3_L, v3_R)
                    nc.vector.tensor_tensor(
                        out=tm[:, :, c0:c1],
                        in0=v[:, :, c0:c1],
                        in1=v[:, :, c0 + 2 : c1 + 2], op=MX,
                    )
                    # DVE: m9 = max(t, v3_C), in place
                    nc.vector.tensor_tensor(
                        out=tm[:, :, c0:c1],
                        in0=tm[:, :, c0:c1],
                        in1=v[:, :, c0 + 1 : c1 + 1], op=MX,
                    )
                for c0, c1 in tailsegs:
                    # DVE: d = x - m9 (<= 0), in place onto tm
                    nc.vector.tensor_tensor(
                        out=tm[:, :, c0:c1],
                        in0=x16[:, 1 : RB + 1, 1 + c0 : 1 + c1],
                        in1=tm[:, :, c0:c1], op=SUB,
                    )
                    # ACT: h = relu(BIG*d + BIG*eps) -> xpe rows RB..2RB
                    nc.scalar.activation(
                        out=xpe[:, RB : 2 * RB, c0:c1], in_=tm[:, :, c0:c1],
                        func=RELU, bias=bigeps[:], scale=BIG,
                    )
                    # DVE: out = min(xp, h), in place onto xpe rows 0..RB
                    nc.vector.tensor_tensor(
                        out=xpe[:, 0:RB, c0:c1],
                        in0=xpe[:, 0:RB, c0:c1],
                        in1=xpe[:, RB : 2 * RB, c0:c1], op=MN,
                    )
                    if do_store and last:
                        nc.gpsimd.dma_start(
                            out=AP(
                                o_d.tensor,
                                out_offset0 + t * BAND_H * out_row_stride + c0,
                                [[RB * out_row_stride, P],
                                 [out_row_stride, RB], [1, c1 - c0]],
                            ),
                            in_=xpe[:, 0:RB, c0:c1],
                        )
            elif do_store and t < 2:
                nc.gpsimd.memset(xpe[:, 0:RB, :], 0.25)

            if do_store and not (do_compute and last):
                # store on the Pool engine's DMA queue (Pool is idle)
                nc.gpsimd.dma_start(
                    out=AP(
                        o_d.tensor,
                        out_offset0 + t * BAND_H * out_row_stride,
                        [[RB * out_row_stride, P], [out_row_stride, RB], [1, W]],
                    ),
                    in_=xpe[:, 0:RB, :],
                )


def _build_program():
    nc = bacc.Bacc(
        "TRN2",
        target_bir_lowering=False,
        debug=False,
        enable_asserts=False,
        num_devices=B,
    )
    xh_d = nc.dram_tensor("xh", [HP2, WP2], F16, kind="ExternalInput").ap()
    o_d = nc.dram_tensor("out", [H, W], F16, kind="ExternalOutput").ap()
    with tile.TileContext(nc) as tc:
        _emit_pipeline(nc, tc, xh_d, o_d, W, 0)
    nc.compile()
    return nc


def _build_timing_program(niter=1, mode="full"):
    """Same pipeline repeated `niter` times by a device-side loop against
    Internal DRAM scratch, with tiny external I/O so transfers are ~free.
    (wall(n2) - wall(n1)) / (n2 - n1) isolates per-pass device time."""
    nc = bacc.Bacc(
        "TRN2",
        target_bir_lowering=False,
        debug=False,
        enable_asserts=False,
        num_devices=B,
    )
    di = nc.dram_tensor("x", [1, 8], F32, kind="ExternalInput").ap()
    do = nc.dram_tensor("out", [1, 8], F32, kind="ExternalOutput").ap()
    xh_d = nc.dram_tensor("xhi", [HP2, WP2], F16, kind="Internal").ap()
    o_d = nc.dram_tensor("oi", [H, W], F16, kind="Internal").ap()
    with tile.TileContext(nc) as tc:
        with tc.tile_pool(name="dummy", bufs=1) as dp:
            dt = dp.tile([1, 8], F32, tag="dummy")
            nc.sync.dma_start(out=dt[:], in_=di[:])
            nc.sync.dma_start(out=do[:], in_=dt[:])
        if niter == 1:
            _emit_pipeline(nc, tc, xh_d, o_d, W, 0, mode)
        else:
            with tc.For_i(0, niter, 1):
                _emit_pipeline(nc, tc, xh_d, o_d, W, 0, mode)
    nc.compile()
    return nc


_NC = None


def _get_program():
    global _NC
    if _NC is None:
        _NC = _build_program()
    return _NC


def kernel(x: np.ndarray) -> np.ndarray:
    x = np.asarray(x, dtype=np.float32)
    assert x.shape == (B, H, W), x.shape
    xh = np.zeros((B, HP2, WP2), dtype=np.float16)
    xh[:, 1 : H + 1, 1 : W + 1] = x.astype(np.float16)
    nc = _get_program()
    in_maps = [{"xh": xh[i]} for i in range(B)]
    res = bass_utils.run_bass_kernel_spmd(nc, in_maps, core_ids=list(range(B)))
    return np.stack(
        [np.asarray(r["out"], dtype=np.float32) for r in res.results], axis=0
    )


# revision 11
# speedup vs baseline: 1.2214x; 1.0623x over previous
"""Trainium2 Bass kernel for nn_DetectionHead (NMS detection head), v4.

Computes, for x[8, 2048, 2048] f32:
    xp  = relu(x - eps)
    xm  = 3x3 hole-excluded neighborhood max of xp (zero padding)
    out = xp * (x > xm)

Sharding: batch (8 images) across the 8 NeuronCores, data parallel.  The
host pads each image with a 1-pixel zero border and converts to fp16
([2050, 2050]); only the fp16 image is shipped.

Math restructure (rel err 1.603e-2 vs the f32 reference, gate 2e-2):
 - Inclusive 3x3 max m9 replaces the hole-excluded max m8: since eps>0,
   x > m8 - eps  <=>  x > m9 - eps  (m9 = max(m8, x)).  Separable:
   v2=max(up,down); v3=max(v2,x) in place; t=max(v3_L,v3_R);
   m9=max(t,v3_C) in place; x16 stays intact for the compare.
 - Tail:  out = min(xp, relu(BIG*d + BIG*eps)),  d = fp16(x - m9) <= 0.
   d is Sterbenz-exact near the threshold (0 mask flips vs f32 compare);
   BIG=2^22 saturates the relu arm to >=215 when the mask is true
   (max xp ~5.2), so min() selects xp exactly.  The mask compare runs on
   the Activation engine (relu w/ scale+bias), not the DVE.

Engine budget per band ([128,4,~2050] ops, cost model):
 - DVE  (2x fp16): v2, v3, t, m9, min = 5 x 4.33us = 21.6us
 - PE:  d = I@x + (-I)@m9 -> PSUM f32 (exact compare), 32 mm = ~9.5us
 - ACT:            xp, h(PSUM)        = ~15us
 - DMA: load 6 rows/part fp16 (9.5us, SP queue), store (6.3us, Pool queue)
Band 0 is column-split so compute starts after half the load; h/min run
per row so the last band's stores overlap compute.

Per-core pipeline, full-width row bands (band t = image rows [512t, 512t+512),
partition p covers padded rows 512t+4p .. 512t+4p+5).
"""

import numpy as np

import concourse.bacc as bacc
import concourse.mybir as mybir
import concourse.tile as tile
from concourse import bass_utils
from concourse.ap import AP

EPS = 0.01
B, H, W = 8, 2048, 2048
HP2, WP2 = H + 2, W + 2
P = 128
RB = 4
BAND_H = RB * P           # 512
NBAND = H // BAND_H       # 4
SB = RB + 2               # 6
HL = W // 2               # 1024
F32 = mybir.dt.float32
F16 = mybir.dt.float16
MX = mybir.AluOpType.max
MN = mybir.AluOpType.min
SUB = mybir.AluOpType.subtract
RELU = mybir.ActivationFunctionType.Relu
BIG = float(2.0 ** 22)
BIGEPS = float(np.float32(0.01) * np.float32(BIG))


def _emit_pipeline(nc, tc, xh_d, o_d, out_row_stride, out_offset0, mode="full"):
    do_load = mode in ("full", "dmaonly", "loadonly")
    do_store = mode in ("full", "dmaonly", "storeonly")
    do_compute = mode in ("full", "nodma")
    with (
        tc.tile_pool(name="io16", bufs=2) as io16,
        tc.tile_pool(name="wv", bufs=2) as wv,
        tc.tile_pool(name="wt", bufs=2) as wt,
        tc.tile_pool(name="wpe", bufs=2) as wpe,
        tc.tile_pool(name="cst", bufs=1) as cst,
        tc.tile_pool(name="psum", bufs=2, space="PSUM") as psum,
    ):
        negeps = cst.tile([P, 1], F32, tag="negeps", name="negeps")
        bigeps = cst.tile([P, 1], F32, tag="bigeps", name="bigeps")
        nc.vector.memset(negeps[:], -EPS)
        nc.vector.memset(bigeps[:], BIGEPS)
        # +-identity weights for the PE subtraction d = I@x + (-I)@m9
        ident = cst.tile([P, P], F16, tag="ident", name="ident")
        nident = cst.tile([P, P], F16, tag="nident", name="nident")
        for w, val in ((ident, 1.0), (nident, -1.0)):
            nc.vector.memset(w[:], val)
            nc.gpsimd.affine_select(
                out=w[:], in_=w[:], pattern=[[1, P]],
                channel_multiplier=-1, base=0,
                compare_op=mybir.AluOpType.is_equal, fill=0.0,
            )
        for t in range(NBAND):
            first, last = (t == 0), (t == NBAND - 1)
            x16 = io16.tile([P, SB, WP2], F16, tag="x16", name="x16")
            v = wv.tile([P, RB, WP2], F16, tag="v", name="v")
            tm = wt.tile([P, RB, W], F16, tag="tm", name="tm")
            xpe = wpe.tile([P, 2 * RB, W], F16, tag="xpe", name="xpe")

            if do_load:
                # padded rows 512t+4p .. 512t+4p+5
                load_segs = [(0, HL + 2), (HL + 2, WP2)] if first else [(0, WP2)]
                for c0, c1 in load_segs:
                    nc.sync.dma_start(
                        out=x16[:, :, c0:c1],
                        in_=AP(
                            xh_d.tensor,
                            t * BAND_H * WP2 + c0,
                            [[RB * WP2, P], [WP2, SB], [1, c1 - c0]],
                        ),
                    )
            elif do_compute or t < 2:
                nc.gpsimd.memset(x16[:], 0.25)

            if do_compute:
                # segments: v2/v3 cols (padded), m9/t + xp out cols (image),
                # tail out cols (image)
                if first:
                    vsegs = [(0, HL + 2), (HL, WP2)]
                    msegs = [(0, HL), (HL, W)]
                else:
                    vsegs = [(0, WP2)]
                    msegs = [(0, W)]

                for c0, c1 in msegs:
                    # ACT: xp = relu(x - eps) -> xpe rows 0..RB
                    nc.scalar.activation(
                        out=xpe[:, 0:RB, c0:c1],
                        in_=x16[:, 1 : RB + 1, 1 + c0 : 1 + c1],
                        func=RELU, bias=negeps[:],
                    )
                for c0, c1 in vsegs:
                    # DVE: v2 = max(up, down)
                    nc.vector.tensor_tensor(
                        out=v[:, :, c0:c1],
                        in0=x16[:, 0:RB, c0:c1],
                        in1=x16[:, 2:SB, c0:c1], op=MX,
                    )
                    # DVE: v3 = max(v2, center), in place
                    nc.vector.tensor_tensor(
                        out=v[:, :, c0:c1],
                        in0=v[:, :, c0:c1],
                        in1=x16[:, 1 : RB + 1, c0:c1], op=MX,
                    )
                for c0, c1 in msegs:
                    # DVE: t = max(v3_L, v3_R)
                    nc.vector.tensor_tensor(
                        out=tm[:, :, c0:c1],
                        in0=v[:, :, c0:c1],
                        in1=v[:, :, c0 + 2 : c1 + 2], op=MX,
                    )
                    # DVE: m9 = max(t, v3_C), in place
                    nc.vector.tensor_tensor(
                        out=tm[:, :, c0:c1],
                        in0=tm[:, :, c0:c1],
                        in1=v[:, :, c0 + 1 : c1 + 1], op=MX,
                    )
                for r in range(RB):
                    # PE: d_r = x_r - m9_r -> PSUM (f32, exact compare)
                    dp = psum.tile([P, W], F32, tag="dpsum", name="dpsum")
                    for c in range(0, W, 512):
                        nc.tensor.matmul(
                            out=dp[:, c : c + 512], lhsT=ident[:],
                            rhs=x16[:, 1 + r, 1 + c : 1 + c + 512],
                            start=True, stop=False,
                        )
                        nc.tensor.matmul(
                            out=dp[:, c : c + 512], lhsT=nident[:],
                            rhs=tm[:, r, c : c + 512],
                            start=False, stop=True,
                        )
                    # ACT: h_r = relu(BIG*d_r + BIG*eps) -> xpe row RB+r
                    nc.scalar.activation(
                        out=xpe[:, RB + r, :], in_=dp[:],
                        func=RELU, bias=bigeps[:], scale=BIG,
                    )
                    # DVE: out_r = min(xp_r, h_r), in place onto xpe row r
                    nc.vector.tensor_tensor(
                        out=xpe[:, r, :],
                        in0=xpe[:, r, :],
                        in1=xpe[:, RB + r, :], op=MN,
                    )
                    if do_store and last:
                        nc.gpsimd.dma_start(
                            out=AP(
                                o_d.tensor,
                                out_offset0
                                + (t * BAND_H + r) * out_row_stride,
                                [[RB * out_row_stride, P], [1, W]],
                            ),
                            in_=xpe[:, r, :],
                        )
            elif do_store and t < 2:
                nc.gpsimd.memset(xpe[:, 0:RB, :], 0.25)

            if do_store and not (do_compute and last):
                # store on the Pool engine's DMA queue (Pool is idle)
                nc.gpsimd.dma_start(
                    out=AP(
                        o_d.tensor,
                        out_offset0 + t * BAND_H * out_row_stride,
                        [[RB * out_row_stride, P], [out_row_stride, RB], [1, W]],
                    ),
                    in_=xpe[:, 0:RB, :],
                )


def _build_program():
    nc = bacc.Bacc(
        "TRN2",
        target_bir_lowering=False,
        debug=False,
        enable_asserts=False,
        num_devices=B,
    )
    xh_d = nc.dram_tensor("xh", [HP2, WP2], F16, kind="ExternalInput").ap()
    o_d = nc.dram_tensor("out", [H, W], F16, kind="ExternalOutput").ap()
    with tile.TileContext(nc) as tc:
        _emit_pipeline(nc, tc, xh_d, o_d, W, 0)
    nc.compile()
    return nc


def _build_timing_program(niter=1, mode="full"):
    """Same pipeline repeated `niter` times by a device-side loop against
    Internal DRAM scratch, with tiny external I/O so transfers are ~free.
    (wall(n2) - wall(n1)) / (n2 - n1) isolates per-pass device time."""
    nc = bacc.Bacc(
        "TRN2",
        target_bir_lowering=False,
        debug=False,
        enable_asserts=False,
        num_devices=B,
    )
    di = nc.dram_tensor("x", [1, 8], F32, kind="ExternalInput").ap()
    do = nc.dram_tensor("out", [1, 8], F32, kind="ExternalOutput").ap()
    xh_d = nc.dram_tensor("xhi", [HP2, WP2], F16, kind="Internal").ap()
    o_d = nc.dram_tensor("oi", [H, W], F16, kind="Internal").ap()
    with tile.TileContext(nc) as tc:
        with tc.tile_pool(name="dummy", bufs=1) as dp:
            dt = dp.tile([1, 8], F32, tag="dummy")
            nc.sync.dma_start(out=dt[:], in_=di[:])
            nc.sync.dma_start(out=do[:], in_=dt[:])
        if niter == 1:
            _emit_pipeline(nc, tc, xh_d, o_d, W, 0, mode)
        else:
            with tc.For_i(0, niter, 1):
                _emit_pipeline(nc, tc, xh_d, o_d, W, 0, mode)
    nc.compile()
    return nc


_NC = None


def _get_program():
    global _NC
    if _NC is None:
        _NC = _build_program()
    return _NC


def kernel(x: np.ndarray) -> np.ndarray:
    x = np.asarray(x, dtype=np.float32)
    assert x.shape == (B, H, W), x.shape
    xh = np.zeros((B, HP2, WP2), dtype=np.float16)
    xh[:, 1 : H + 1, 1 : W + 1] = x.astype(np.float16)
    nc = _get_program()
    in_maps = [{"xh": xh[i]} for i in range(B)]
    res = bass_utils.run_bass_kernel_spmd(nc, in_maps, core_ids=list(range(B)))
    return np.stack(
        [np.asarray(r["out"], dtype=np.float32) for r in res.results], axis=0
    )


# revision 12
# speedup vs baseline: 1.4129x; 1.1568x over previous
"""Trainium2 Bass kernel for nn_DetectionHead (NMS detection head), v4.

Computes, for x[8, 2048, 2048] f32:
    xp  = relu(x - eps)
    xm  = 3x3 hole-excluded neighborhood max of xp (zero padding)
    out = xp * (x > xm)

Sharding: batch (8 images) across the 8 NeuronCores, data parallel.  The
host pads each image with a 1-pixel zero border and converts to fp16
([2050, 2050]); only the fp16 image is shipped.

Math restructure (rel err 1.603e-2 vs the f32 reference, gate 2e-2):
 - Inclusive 3x3 max m9 replaces the hole-excluded max m8: since eps>0,
   x > m8 - eps  <=>  x > m9 - eps  (m9 = max(m8, x)).  Separable:
   v2=max(up,down); v3=max(v2,x) in place; t=max(v3_L,v3_R);
   m9=max(t,v3_C) in place; x16 stays intact for the compare.
 - Tail:  out = min(xp, relu(BIG*d + BIG*eps)),  d = fp16(x - m9) <= 0.
   d is Sterbenz-exact near the threshold (0 mask flips vs f32 compare);
   BIG=2^22 saturates the relu arm to >=215 when the mask is true
   (max xp ~5.2), so min() selects xp exactly.  The mask compare runs on
   the Activation engine (relu w/ scale+bias), not the DVE.

Engine budget per band ([128,4,~2050] ops, cost model):
 - DVE  (2x fp16): v2, v3, t, m9, min = 5 x 4.33us = 21.6us
 - PE:  d = I@x + (-I)@m9 -> PSUM f32 (exact compare), 32 mm = ~9.5us
 - ACT:            xp, h(PSUM)        = ~15us
 - DMA: load 6 rows/part fp16 (9.5us, SP queue), store (6.3us, Pool queue)
Band 0 is column-split so compute starts after half the load; h/min run
per row so the last band's stores overlap compute.

Per-core pipeline, full-width row bands (band t = image rows [512t, 512t+512),
partition p covers padded rows 512t+4p .. 512t+4p+5).
"""

import numpy as np

import concourse.bacc as bacc
import concourse.mybir as mybir
import concourse.tile as tile
from concourse import bass_utils
from concourse.ap import AP

EPS = 0.01
B, H, W = 8, 2048, 2048
HP2, WP2 = H + 2, W + 2
P = 128
RB = 4
BAND_H = RB * P           # 512
NBAND = H // BAND_H       # 4
SB = RB + 2               # 6
HL = W // 2               # 1024
F32 = mybir.dt.float32
F16 = mybir.dt.float16
MX = mybir.AluOpType.max
MN = mybir.AluOpType.min
SUB = mybir.AluOpType.subtract
RELU = mybir.ActivationFunctionType.Relu
BIG = float(2.0 ** 22)
BIGEPS = float(np.float32(0.01) * np.float32(BIG))


def _emit_pipeline(nc, tc, xh_d, o_d, out_row_stride, out_offset0, mode="full"):
    do_load = mode in ("full", "dmaonly", "loadonly")
    do_store = mode in ("full", "dmaonly", "storeonly")
    do_compute = mode in ("full", "nodma")
    with (
        tc.tile_pool(name="io16", bufs=3) as io16,
        tc.tile_pool(name="wv", bufs=2) as wv,
        tc.tile_pool(name="wt", bufs=2) as wt,
        tc.tile_pool(name="wpe", bufs=2) as wpe,
        tc.tile_pool(name="whe", bufs=1) as whe,
        tc.tile_pool(name="cst", bufs=1) as cst,
        tc.tile_pool(name="psum", bufs=2, space="PSUM") as psum,
    ):
        negeps = cst.tile([P, 1], F32, tag="negeps", name="negeps")
        bigeps = cst.tile([P, 1], F32, tag="bigeps", name="bigeps")
        nc.vector.memset(negeps[:], -EPS)
        nc.vector.memset(bigeps[:], BIGEPS)
        # +-identity weights for the PE subtraction d = I@x + (-I)@m9
        ident = cst.tile([P, P], F16, tag="ident", name="ident")
        nident = cst.tile([P, P], F16, tag="nident", name="nident")
        for w, val in ((ident, 1.0), (nident, -1.0)):
            nc.vector.memset(w[:], val)
            nc.gpsimd.affine_select(
                out=w[:], in_=w[:], pattern=[[1, P]],
                channel_multiplier=-1, base=0,
                compare_op=mybir.AluOpType.is_equal, fill=0.0,
            )
        for t in range(NBAND):
            first, last = (t == 0), (t == NBAND - 1)
            x16 = io16.tile([P, SB, WP2], F16, tag="x16", name="x16")
            v = wv.tile([P, RB, WP2], F16, tag="v", name="v")
            tm = wt.tile([P, RB, W], F16, tag="tm", name="tm")
            xpe = wpe.tile([P, RB, W], F16, tag="xpe", name="xpe")
            he = whe.tile([P, RB, W], F16, tag="he", name="he")

            if do_load:
                # padded rows 512t+4p .. 512t+4p+5
                load_segs = [(0, HL + 2), (HL + 2, WP2)] if first else [(0, WP2)]
                for c0, c1 in load_segs:
                    nc.sync.dma_start(
                        out=x16[:, :, c0:c1],
                        in_=AP(
                            xh_d.tensor,
                            t * BAND_H * WP2 + c0,
                            [[RB * WP2, P], [WP2, SB], [1, c1 - c0]],
                        ),
                    )
            elif do_compute or t < 2:
                nc.gpsimd.memset(x16[:], 0.25)

            if do_compute:
                # segments: v2/v3 cols (padded), m9/t + xp out cols (image),
                # tail out cols (image)
                if first:
                    vsegs = [(0, HL + 2), (HL, WP2)]
                    msegs = [(0, HL), (HL, W)]
                elif last:
                    vsegs = [(0, WP2)]
                    msegs = [(0, HL), (HL, W)]
                else:
                    vsegs = [(0, WP2)]
                    msegs = [(0, W)]
                tailsegs = msegs if last else [(0, W)]

                for c0, c1 in msegs:
                    # ACT: xp = relu(x - eps) -> xpe rows 0..RB
                    nc.scalar.activation(
                        out=xpe[:, 0:RB, c0:c1],
                        in_=x16[:, 1 : RB + 1, 1 + c0 : 1 + c1],
                        func=RELU, bias=negeps[:],
                    )
                for c0, c1 in vsegs:
                    # DVE: v2 = max(up, down)
                    nc.vector.tensor_tensor(
                        out=v[:, :, c0:c1],
                        in0=x16[:, 0:RB, c0:c1],
                        in1=x16[:, 2:SB, c0:c1], op=MX,
                    )
                    # DVE: v3 = max(v2, center), in place
                    nc.vector.tensor_tensor(
                        out=v[:, :, c0:c1],
                        in0=v[:, :, c0:c1],
                        in1=x16[:, 1 : RB + 1, c0:c1], op=MX,
                    )
                for c0, c1 in msegs:
                    # DVE: t = max(v3_L, v3_R)
                    nc.vector.tensor_tensor(
                        out=tm[:, :, c0:c1],
                        in0=v[:, :, c0:c1],
                        in1=v[:, :, c0 + 2 : c1 + 2], op=MX,
                    )
                    # DVE: m9 = max(t, v3_C), in place
                    nc.vector.tensor_tensor(
                        out=tm[:, :, c0:c1],
                        in0=tm[:, :, c0:c1],
                        in1=v[:, :, c0 + 1 : c1 + 1], op=MX,
                    )
                for s0, s1 in tailsegs:
                    for r in range(RB):
                        # PE: d_r = x_r - m9_r -> PSUM (f32, exact compare)
                        dp = psum.tile([P, W // len(tailsegs)], F32,
                                       tag="dpsum", name="dpsum")
                        for ci, c in enumerate(range(s0, s1, 512)):
                            nc.tensor.matmul(
                                out=dp[:, ci * 512 : (ci + 1) * 512],
                                lhsT=ident[:],
                                rhs=x16[:, 1 + r, 1 + c : 1 + c + 512],
                                start=True, stop=False,
                            )
                            nc.tensor.matmul(
                                out=dp[:, ci * 512 : (ci + 1) * 512],
                                lhsT=nident[:],
                                rhs=tm[:, r, c : c + 512],
                                start=False, stop=True,
                            )
                        # ACT: h_r = relu(BIG*d_r + BIG*eps) -> he row r
                        nc.scalar.activation(
                            out=he[:, r, s0:s1], in_=dp[:],
                            func=RELU, bias=bigeps[:], scale=BIG,
                        )
                        # DVE: out_r = min(xp_r, h_r), in place onto xpe row r
                        nc.vector.tensor_tensor(
                            out=xpe[:, r, s0:s1],
                            in0=xpe[:, r, s0:s1],
                            in1=he[:, r, s0:s1], op=MN,
                        )
                        if do_store and last:
                            nc.gpsimd.dma_start(
                                out=AP(
                                    o_d.tensor,
                                    out_offset0
                                    + (t * BAND_H + r) * out_row_stride + s0,
                                    [[RB * out_row_stride, P], [1, s1 - s0]],
                                ),
                                in_=xpe[:, r, s0:s1],
                            )
            elif do_store and t < 2:
                nc.gpsimd.memset(xpe[:], 0.25)

            if do_store and not (do_compute and last):
                # store on the Pool engine's DMA queue (Pool is idle)
                nc.gpsimd.dma_start(
                    out=AP(
                        o_d.tensor,
                        out_offset0 + t * BAND_H * out_row_stride,
                        [[RB * out_row_stride, P], [out_row_stride, RB], [1, W]],
                    ),
                    in_=xpe[:],
                )


def _build_program():
    nc = bacc.Bacc(
        "TRN2",
        target_bir_lowering=False,
        debug=False,
        enable_asserts=False,
        num_devices=B,
    )
    xh_d = nc.dram_tensor("xh", [HP2, WP2], F16, kind="ExternalInput").ap()
    o_d = nc.dram_tensor("out", [H, W], F16, kind="ExternalOutput").ap()
    with tile.TileContext(nc) as tc:
        _emit_pipeline(nc, tc, xh_d, o_d, W, 0)
    nc.compile()
    return nc


def _build_timing_program(niter=1, mode="full"):
    """Same pipeline repeated `niter` times by a device-side loop against
    Internal DRAM scratch, with tiny external I/O so transfers are ~free.
    (wall(n2) - wall(n1)) / (n2 - n1) isolates per-pass device time."""
    nc = bacc.Bacc(
        "TRN2",
        target_bir_lowering=False,
        debug=False,
        enable_asserts=False,
        num_devices=B,
    )
    di = nc.dram_tensor("x", [1, 8], F32, kind="ExternalInput").ap()
    do = nc.dram_tensor("out", [1, 8], F32, kind="ExternalOutput").ap()
    xh_d = nc.dram_tensor("xhi", [HP2, WP2], F16, kind="Internal").ap()
    o_d = nc.dram_tensor("oi", [H, W], F16, kind="Internal").ap()
    with tile.TileContext(nc) as tc:
        with tc.tile_pool(name="dummy", bufs=1) as dp:
            dt = dp.tile([1, 8], F32, tag="dummy")
            nc.sync.dma_start(out=dt[:], in_=di[:])
            nc.sync.dma_start(out=do[:], in_=dt[:])
        if niter == 1:
            _emit_pipeline(nc, tc, xh_d, o_d, W, 0, mode)
        else:
            with tc.For_i(0, niter, 1):
                _emit_pipeline(nc, tc, xh_d, o_d, W, 0, mode)
    nc.compile()
    return nc


_NC = None


def _get_program():
    global _NC
    if _NC is None:
        _NC = _build_program()
    return _NC


def kernel(x: np.ndarray) -> np.ndarray:
    x = np.asarray(x, dtype=np.float32)
    assert x.shape == (B, H, W), x.shape
    xh = np.zeros((B, HP2, WP2), dtype=np.float16)
    xh[:, 1 : H + 1, 1 : W + 1] = x.astype(np.float16)
    nc = _get_program()
    in_maps = [{"xh": xh[i]} for i in range(B)]
    res = bass_utils.run_bass_kernel_spmd(nc, in_maps, core_ids=list(range(B)))
    return np.stack(
        [np.asarray(r["out"], dtype=np.float32) for r in res.results], axis=0
    )
